# revision 1
# baseline (speedup 1.0000x reference)
"""Trainium2 Bass kernel for nn_BasicFlow (sparse window attention flow).

Sharding: pure data-parallel over batch B=8 -> one image pair per NeuronCore.

Device (per core):
  - PE warm-up: a psum-only matmul accumulation group runs during the input
    DMA wait so the p-state ramp (0.65/1.2 -> 2.4 GHz) completes for free.
  - 4x conv3x3 (128->128ch, 96x96) as fp8e4m3 DoubleRow matmuls over a
    host-pre-padded flat feature map at 0.5 cycles/row. Default fp8x2 mode:
    features are split hi/lo (fh = fp8(f), fl = fp8(32*(f-fh))) and each
    DoubleRow pair contracts one tap's hi term (weights 64*W) together with
    its lo term (weights 2*W) - 9 pair-matmuls per 4-row strip, compensating
    feature quantization exactly. Weights are pre-scaled x64 into e4m3
    range; the x64 bias is added at the psum->SBUF copy and the host divides
    the correlation volume by 64^2.
  - q-conv outputs drain into paired window-major layouts qh[c,wx,block,64]
    (block h = rows 4h..4h+8, overlapping): consecutive block pairs form a
    CONTIGUOUS 128-wide stationary [A|B] holding BOTH row-shift variants'
    q-windows (the stationary operand must be 1-D contiguous - walrus "RHS
    AP one free dim"), and a col-shifted copy serves the two col phases.
  - correlations as E/O pair matmuls: one matmul streams 32 shared
    k-columns (strided MOVING reads from the wrap-extended [128,100,100]
    bf16 k tensors - multi-dim free is allowed for the moving side) into
    both variants' outputs at once, HALVING corr PE time vs one matmul per
    window. The host reassembles [vd, win, q, k] from the A/B halves.
  - corr direction 0 is interleaved between the conv pairs (each rx burst
    separately, staying inside the 8-bank psum runway) so its psum drains
    and output DMA overlap the q2/k0 convs, and the direction-0 q layouts
    can be buffer-recycled for direction 1.
Host: softmax/flow/splice/bilinear tail (numpy, ~1% of FLOPs).
Cost-model exec: ~104.4us/core (baseline: 258.5us), rel err 1.12e-2 (cpu
input realization) / 1.38e-2 (axon realization), gate 2e-2.
Other BASSFLOW_MODEs: fp8 1-set (faster, but up to 2.25e-2 - unsafe),
bf16 (~1.5e-3).
"""

import os

# recover wedged NeuronCores at NRT init (observed transient
# NRT_EXEC_UNIT_UNRECOVERABLE; reset-on-load clears it)
os.environ.setdefault("NEURON_RT_RESET_CORES", "1")

import numpy as np
import ml_dtypes

import concourse.bass as bass
import concourse.bacc as bacc
import concourse.tile as tile
import concourse.mybir as mybir
from concourse import bass_utils
from concourse.bass import AP

F32 = mybir.dt.float32
BF16 = mybir.dt.bfloat16
FP8 = mybir.dt.float8e4

B = 8
DIM = 128
H = W = 96
P = 8
UP = 4
SCALE = DIM ** -0.5
S1 = S2 = H // P          # 12 windows per axis
NW = S1 * S2              # 144 windows
NV = 8                    # 4 shift variants x 2 directions

# padded feature map: row 0 + 96 image rows + row 97 border + 3 zero tail
# rows (DoubleRow zero-slot reads); cols: 1 border + 96 + 1 border
FR = 101                  # fpad rows
FC = 98                   # fpad cols
EXT = 100                 # extended conv-out tensor: 96 + 4 wrap rows/cols

WSCALE = 64.0             # fp8 weight pre-scale; corr is WSCALE^2 too big
LSCALE = 32.0             # feature-residual pre-scale (fp8x2 mode)

# conv taps as DoubleRow pairs: (slot0 tap, slot1 tap, ifmap delta)
# tap (dy, dx) reads flat offset dy*FC + dx relative to the strip base.
# pair 4 slot1 is a zero-weight dummy (reads in-bounds zero-padded rows).
_PAIRS = [((0, 0), (0, 1)), ((0, 2), (1, 0)), ((1, 1), (1, 2)),
          ((2, 0), (2, 1)), ((2, 2), None)]
_PAIR_OFF = [dy * FC + dx for ((dy, dx), _) in _PAIRS]
_PAIR_D = []
for (t0, t1) in _PAIRS:
    if t1 is None:
        _PAIR_D.append(FC)    # dummy slot: any in-bounds stride
    else:
        _PAIR_D.append((t1[0] * FC + t1[1]) - (t0[0] * FC + t0[1]))

# fp8x2 is the default: the final rel-err depends on the RNG realization of
# setup_inputs (jax PRNG differs between the axon and cpu platforms!), and
# 1-set fp8 measured 1.66e-2 (cpu inputs) but 2.25e-2 (axon inputs) - over
# the 2e-2 gate. fp8x2 measures 1.10e-2 / 1.33e-2 - safe on both.
MODE = os.environ.get("BASSFLOW_MODE", "fp8x2")   # fp8 | fp8x2 | bf16

_COMPILED = {}


# --------------------------------------------------------------------------
# Device kernel
# --------------------------------------------------------------------------

def _build_device(mode):
    nc = bacc.Bacc("TRN2", target_bir_lowering=False, debug=False,
                   num_devices=8)

    fdt = BF16 if mode == "bf16" else FP8
    f0h_d = nc.dram_tensor("f0h", [DIM, FR * FC], fdt, kind="ExternalInput")
    f2h_d = nc.dram_tensor("f2h", [DIM, FR * FC], fdt, kind="ExternalInput")
    if mode == "fp8x2":
        f0l_d = nc.dram_tensor("f0l", [DIM, FR * FC], FP8,
                               kind="ExternalInput")
        f2l_d = nc.dram_tensor("f2l", [DIM, FR * FC], FP8,
                               kind="ExternalInput")
    if mode == "bf16":
        wq_d = nc.dram_tensor("wq", [DIM, 9, DIM], BF16, kind="ExternalInput")
        wk_d = nc.dram_tensor("wk", [DIM, 9, DIM], BF16, kind="ExternalInput")
    elif mode == "fp8x2":
        # per tap: slot0 = 64*W (pairs with fh), slot1 = 2*W (pairs with fl)
        wq_d = nc.dram_tensor("wq", [DIM, 9, 2, DIM], FP8,
                              kind="ExternalInput")
        wk_d = nc.dram_tensor("wk", [DIM, 9, 2, DIM], FP8,
                              kind="ExternalInput")
    else:
        wq_d = nc.dram_tensor("wq", [DIM, 5, 2, DIM], FP8,
                              kind="ExternalInput")
        wk_d = nc.dram_tensor("wk", [DIM, 5, 2, DIM], FP8,
                              kind="ExternalInput")
    bq_d = nc.dram_tensor("bq", [DIM, 1], F32, kind="ExternalInput")
    bk_d = nc.dram_tensor("bk", [DIM, 1], F32, kind="ExternalInput")
    # raw correlation volumes (x WSCALE^2) in E/O pair-group form:
    # [d*2+rxi, pairhalf*64+q_pixel, wx*24+wy*2+eo, k_half] - the host
    # reassembles [vd, win, q, k] from the A/B partition halves
    corr_d = nc.dram_tensor("corr", [4, 2 * P * P, 2 * NW, 32], BF16,
                            kind="ExternalOutput")

    with tile.TileContext(nc) as tc:
        with (
            tc.tile_pool(name="const", bufs=1) as constp,
            tc.tile_pool(name="big", bufs=2) as bigp,
            tc.tile_pool(name="qk", bufs=1) as qkp,
            tc.tile_pool(name="psum", bufs=8, space="PSUM") as psump,
        ):
            if mode == "bf16":
                wq_sb = constp.tile([DIM, 9, DIM], BF16, tag="wq")
                wk_sb = constp.tile([DIM, 9, DIM], BF16, tag="wk")
            elif mode == "fp8x2":
                wq_sb = constp.tile([DIM, 9, 2, DIM], FP8, tag="wq")
                wk_sb = constp.tile([DIM, 9, 2, DIM], FP8, tag="wk")
            else:
                wq_sb = constp.tile([DIM, 5, 2, DIM], FP8, tag="wq")
                wk_sb = constp.tile([DIM, 5, 2, DIM], FP8, tag="wk")
            bq_sb = constp.tile([DIM, 1], F32, tag="bq")
            bk_sb = constp.tile([DIM, 1], F32, tag="bk")
            # padded feature maps in row-chunks (small first chunk -> conv
            # starts ASAP). Two HWDGE queues, each in strict need-order:
            #   SP : f0 hi(/lo) chunk pairs (first conv)
            #   Act: wq, bq, then f2 chunk pairs, then wk, bk
            # fp8x2 keeps hi and lo pads in ONE tile so a DoubleRow pair can
            # contract a tap's hi and lo terms together (slot delta = FR*FC).
            NSL = 2 if mode == "fp8x2" else 1
            if mode == "fp8x2":
                CHUNKS = [(0, 14), (14, 30), (30, 54), (54, 78), (78, FR)]
            else:
                CHUNKS = [(0, 14), (14, 42), (42, 72), (72, FR)]
            fpads = {}
            f0srcs = [("f0", 0, f0h_d)]
            f2srcs = [("f2", 0, f2h_d)]
            if mode == "fp8x2":
                f0srcs.append(("f0", 1, f0l_d))
                f2srcs.append(("f2", 1, f2l_d))
            for nmk in ("f0", "f2"):
                fpads[nmk] = bigp.tile([DIM, NSL, FR * FC], fdt,
                                       tag="big", name=nmk)
            nc.scalar.dma_start(wq_sb[:], wq_d[:])
            nc.scalar.dma_start(bq_sb[:], bq_d[:])
            # f0 chunks stream on SP/HWDGE; everything needed later (f2
            # chunks, wk, bk) goes on the gpsimd SWDGE queue, head-delayed
            # by a WAW dependency on a 1-element memset emitted mid-q0-conv
            # so these transfers cannot grab the serial DMA device ahead of
            # the f0 chunks the first conv is consuming.
            for (r0, r1) in CHUNKS:
                for nmk, sl, src_d in f0srcs:
                    nc.sync.dma_start(fpads[nmk][:, sl, r0 * FC:r1 * FC],
                                      src_d[:, r0 * FC:r1 * FC])

            def emit_deferred():
                # the WAW dep on this late-running DVE memset holds the
                # whole SWDGE queue back until the f0 chunks are through
                nc.vector.memset(fpads["f2"][:, 0, 0:8], 0.0)
                for (r0, r1) in CHUNKS:
                    for nmk, sl, src_d in f2srcs:
                        nc.gpsimd.dma_start(
                            fpads[nmk][:, sl, r0 * FC:r1 * FC],
                            src_d[:, r0 * FC:r1 * FC])
                nc.gpsimd.dma_start(wk_sb[:], wk_d[:])
                nc.gpsimd.dma_start(bk_sb[:], bk_d[:])

            # k-side: extended [100, 100] tensors (wrap rows/cols duplicated)
            # read directly as strided MOVING matmul operands.
            # q-side: STATIONARY operands must be 1-D contiguous, so build two
            # window-major layouts per q tensor (col phase 0 and 4), each with
            # ly extended to 12 so both row phases are contiguous slices:
            #   qw[c, wy, wx, ly, lx] = q[c, (wy*8+ly)%96, (wx*8+rx+lx)%96]
            k0e = qkp.tile([DIM, EXT, EXT], BF16, tag="k0")
            k2e = qkp.tile([DIM, EXT, EXT], BF16, tag="k2")

            # warm up the PE p-state during the input-DMA wait: dummy matmuls
            # on broadcast const-pool operands (no data dependencies) keep the
            # engine busy so the 3us ramp to full clock completes before the
            # first conv strip
            wps = psump.tile([DIM, 512], F32, tag="ps")
            cone = nc.const_aps.scalar_like(1.0, wps[:, 0:1], BF16)
            warm_l = cone.broadcast_to([DIM, DIM])
            warm_r = cone.broadcast_to([DIM, 512])
            NWARM = 8 if mode != "fp8x2" else 10
            for i in range(NWARM):
                # one accumulation group: no inter-matmul semaphores, so the
                # PE stays gaplessly busy and the p-state ramp is not reset
                nc.tensor.matmul(wps[:], warm_l, warm_r,
                                 start=(i == 0), stop=(i == NWARM - 1))
            # paired-q layout qh[c, wx, block, 64], block h = window rows
            # 4h..4h+8 (h in [0,25), 24 = wrap copy of block 0): consecutive
            # block pairs give a CONTIGUOUS 128-wide stationary [A|B] so one
            # matmul streams 32 shared k-columns into BOTH row-shift
            # variants' outputs at once - halving corr PE time. The two q
            # directions REUSE the same buffers (bufs=1 tags): qw0 layouts
            # die after the interleaved corr-d0 pass, before qw2's drains.

            NRT = H // 4                 # 24 strips of 4 output rows

            def pair_ap(fp_sb, rt, pr):
                """fp8 1-set: DoubleRow slots = two taps of the hi pad."""
                base = fp_sb[:]
                pitch = base.ap[0][0]
                off = rt * 4 * FC + _PAIR_OFF[pr]
                return AP(tensor=base.tensor, offset=off,
                          ap=[[pitch, DIM], [_PAIR_D[pr], 2], [1, 4 * FC]])

            def hilo_ap(fp_sb, rt, t):
                """fp8x2: DoubleRow slots = same tap's hi and lo terms."""
                base = fp_sb[:]
                pitch = base.ap[0][0]
                dy, dx = divmod(t, 3)
                off = rt * 4 * FC + dy * FC + dx
                return AP(tensor=base.tensor, offset=off,
                          ap=[[pitch, DIM], [FR * FC, 2], [1, 4 * FC]])

            def tap_ap(fp_sb, rt, t):
                base = fp_sb[:]
                pitch = base.ap[0][0]
                dy, dx = divmod(t, 3)
                off = rt * 4 * FC + dy * FC + dx
                return AP(tensor=base.tensor, offset=off,
                          ap=[[pitch, DIM], [1, 4 * FC]])

            def conv_mms(ps, fh_name, w_sb, rt):
                fh = fpads[fh_name]
                if mode == "bf16":
                    for t in range(9):
                        nc.tensor.matmul(ps[:], w_sb[:, t, :],
                                         tap_ap(fh, rt, t),
                                         start=(t == 0), stop=(t == 8))
                elif mode == "fp8x2":
                    for t in range(9):
                        nc.tensor.matmul(
                            ps[:], w_sb[:, t], hilo_ap(fh, rt, t),
                            start=(t == 0), stop=(t == 8),
                            perf_mode=mybir.MatmulPerfMode.DoubleRow)
                else:
                    for i in range(5):
                        nc.tensor.matmul(
                            ps[:], w_sb[:, i], pair_ap(fh, rt, i),
                            start=(i == 0), stop=(i == 4),
                            perf_mode=mybir.MatmulPerfMode.DoubleRow)

            def drain(d_ap, s_ap, b_sb, eng):
                if eng == 0:
                    nc.scalar.activation(
                        d_ap, s_ap, mybir.ActivationFunctionType.Identity,
                        bias=b_sb[:])
                else:
                    nc.vector.tensor_scalar_add(d_ap, s_ap, b_sb[:])

            def strip_src(ps, order_wx_first):
                """psum [128, 4*FC] as [wx, r, lx] or [r, cols]."""
                base = ps[:]
                pitch = base.ap[0][0]
                if order_wx_first:
                    return AP(tensor=base.tensor, offset=base.offset,
                              ap=[[pitch, DIM], [P, S2], [FC, 4], [1, P]])
                return base.rearrange("p (a b) -> p a b", b=FC)[:, :, 0:96]

            def conv_k(dst, fh_name, w_sb, b_sb, ci):
                for rt in range(NRT):
                    ps = psump.tile([DIM, 4 * FC], F32, tag="ps")
                    conv_mms(ps, fh_name, w_sb, rt)
                    src = strip_src(ps, False)
                    drain(dst[:, rt * 4:rt * 4 + 4, 0:96], src, b_sb,
                          (rt + ci) % 2)
                    if rt == 0:          # wrap rows 96..99 = rows 0..3
                        drain(dst[:, 96:100, 0:96], src, b_sb, (rt + ci + 1) % 2)
                # wrap cols 96..99 = cols 0..3 (incl. wrap-row corner)
                nc.gpsimd.tensor_copy(dst[:, :, 96:100], dst[:, :, 0:4])

            NB = 25                       # qh blocks per wx (24 + wrap dup)

            def qh_dst(x0t, blk, e0):
                """qh[:, :, blk, e0:e0+32] shaped [128, 12wx, 4r, 8lx]."""
                base = x0t[:]
                pitch = base.ap[0][0]
                off = blk * 64 + e0
                return AP(tensor=base.tensor, offset=off,
                          ap=[[pitch, DIM], [NB * 64, S2], [P, 4], [1, P]])

            def conv_q(fh_name, w_sb, b_sb, ci, dirn):
                x0 = qkp.tile([DIM, S2, NB, 2 * 32], BF16, tag="qh_x0",
                              name=f"qh{dirn}_x0")
                x4 = qkp.tile([DIM, S2, NB, 2 * 32], BF16, tag="qh_x4",
                              name=f"qh{dirn}_x4")
                for rt in range(NRT):
                    if dirn == 0 and rt == 6:
                        emit_deferred()
                    ps = psump.tile([DIM, 4 * FC], F32, tag="ps")
                    conv_mms(ps, fh_name, w_sb, rt)
                    src = strip_src(ps, True)     # [128, wx, r, lx]
                    # strip rt = 4-row segment rt: first half of block rt,
                    # second half of block rt-1 (blocks overlap by 4 rows)
                    dsts = [qh_dst(x0, rt, 0),
                            qh_dst(x0, rt - 1 if rt else 23, 32)]
                    if rt <= 1:           # wrap block 24 = rows 96..104
                        dsts.append(qh_dst(x0, 24, 32 * rt))
                    for j, dd in enumerate(dsts):
                        drain(dd, src, b_sb, (rt + ci + j) % 2)
                # x4 = col-shifted view of x0, split across DVE + gpsimd
                x0v = x0[:].rearrange("p w b (r l) -> p w b r l", l=P)
                x4v = x4[:].rearrange("p w b (r l) -> p w b r l", l=P)
                nc.vector.tensor_copy(x4v[:, :, :, :, 0:4],
                                      x0v[:, :, :, :, 4:8])
                nc.gpsimd.tensor_copy(x4v[:, 0:S2 - 1, :, :, 4:8],
                                      x0v[:, 1:S2, :, :, 0:4])
                nc.gpsimd.tensor_copy(x4v[:, S2 - 1, :, :, 4:8],
                                      x0v[:, 0, :, :, 0:4])
                return {0: x0, 4: x4}

            def corr_dir(d, qh, ks, rxs=(0, 4)):
                """rx groups of one direction: per (wx, wy) an E and an
                O matmul, each streaming 32 shared k-columns into a 128-wide
                [A|B] stationary pair. Slot order: wx*24 + wy*2 + eo."""
                for rxi, rx in [(0, 0), (1, 4)]:
                    if rx not in rxs:
                        continue
                    g = d * 2 + rxi
                    qp = qh[rx]
                    st = bigp.tile([2 * P * P, 2 * NW, 32], BF16, tag="st",
                                   name=f"st{g}")
                    # d1 runs last: fine chunks keep the serial DMA device
                    # fed as slots drain instead of backlogging past PE-end
                    bounds = (list(range(0, 289, 32)) if d == 1
                              else [0, 96, 192, 288])
                    cnt = 0
                    ps = None
                    for wx in range(S2):
                        for wy in range(S1):
                            for eo in range(2):
                                if cnt % 16 == 0:
                                    ps = psump.tile([2 * P * P, 16, 32], F32,
                                                    tag="ps")
                                if eo == 0:
                                    b0, r0 = 2 * wy, P * wy + 4
                                else:
                                    b0 = 2 * wy - 1 if wy else 23
                                    r0 = P * wy
                                nc.tensor.matmul(
                                    ps[:, cnt % 16, :],
                                    qp[:, wx, b0:b0 + 2, :],
                                    ks[:, r0:r0 + 4,
                                       rx + P * wx:rx + P * wx + P],
                                    start=True, stop=True)
                                cnt += 1
                                if cnt % 16 == 0:
                                    dst = st[:, cnt - 16:cnt, :]
                                    if (cnt // 16) % 2 == 0:
                                        nc.scalar.copy(dst, ps[:])
                                    else:
                                        nc.vector.tensor_copy(dst, ps[:])
                                if cnt in bounds:
                                    p0 = bounds[bounds.index(cnt) - 1]
                                    nc.sync.dma_start(
                                        corr_d[g, :, p0:cnt, :],
                                        st[:, p0:cnt, :])

            # interleave: corr-d0 runs between the conv pairs so its drains
            # and output DMA overlap the q2/k0 convs (and the qh buffers of
            # direction 0 can be recycled for direction 2)
            # corr-d0 split into its two rx bursts around the q2 conv:
            # an 18-bank burst stays inside the 8-bank psum runway (no
            # drain-rate stall), a contiguous 36-bank one does not
            qh0 = conv_q("f0", wq_sb, bq_sb, 0, 0)
            conv_k(k2e, "f2", wk_sb, bk_sb, 1)
            corr_dir(0, qh0, k2e, rxs=(0,))
            qh2 = conv_q("f2", wq_sb, bq_sb, 2, 2)
            corr_dir(0, qh0, k2e, rxs=(4,))
            conv_k(k0e, "f0", wk_sb, bk_sb, 3)
            corr_dir(1, qh2, k0e)

    nc.compile()
    return nc


# --------------------------------------------------------------------------
# Host-side prep + run
# --------------------------------------------------------------------------

def _pack_weights(w, scale, dt):
    """w: (out, in, 3, 3) -> (in, 5, 2, out) DoubleRow tap pairs, x scale."""
    w = np.asarray(w, np.float32) * scale
    pk = np.zeros((DIM, 5, 2, DIM), np.float32)
    for p, (t0, t1) in enumerate(_PAIRS):
        pk[:, p, 0, :] = w[:, :, t0[0], t0[1]].T
        if t1 is not None:
            pk[:, p, 1, :] = w[:, :, t1[0], t1[1]].T
    return np.ascontiguousarray(pk).astype(dt)


def _pack_weights_hilo(w, dt):
    """fp8x2: (out, in, 3, 3) -> (in, 9, 2, out); per tap slot0 = 64*W
    (hi features), slot1 = 2*W (x32-scaled feature residuals)."""
    w = np.asarray(w, np.float32)
    pk = np.zeros((DIM, 9, 2, DIM), np.float32)
    for t in range(9):
        dy, dx = divmod(t, 3)
        pk[:, t, 0, :] = (WSCALE * w)[:, :, dy, dx].T
        pk[:, t, 1, :] = ((WSCALE / LSCALE) * w)[:, :, dy, dx].T
    return np.ascontiguousarray(pk).astype(dt)


def _pack_weights_taps(w, scale, dt):
    """bf16 mode: (out, in, 3, 3) -> (in, 9, out) x scale."""
    w = np.asarray(w, np.float32) * scale
    pk = w.transpose(1, 2, 3, 0).reshape(DIM, 9, DIM)
    return np.ascontiguousarray(pk).astype(dt)


def _pad_feat(f, dt):
    """f: (128, 96, 96) f32 -> flat (128, FR*FC) padded, quantized to dt."""
    fp = np.zeros((DIM, FR, FC), np.float32)
    fp[:, 1:97, 1:97] = f
    return np.ascontiguousarray(fp.reshape(DIM, FR * FC)).astype(dt)


def _run_device(feat0, feat2, wq, bq, wk, bk):
    mode = MODE
    if mode not in _COMPILED:
        _COMPILED[mode] = _build_device(mode)
    nc = _COMPILED[mode]

    bf = ml_dtypes.bfloat16
    e4 = ml_dtypes.float8_e4m3
    if mode == "bf16":
        wqp = _pack_weights_taps(wq, WSCALE, bf)
        wkp = _pack_weights_taps(wk, WSCALE, bf)
    elif mode == "fp8x2":
        wqp = _pack_weights_hilo(wq, e4)
        wkp = _pack_weights_hilo(wk, e4)
    else:
        wqp = _pack_weights(wq, WSCALE, e4)
        wkp = _pack_weights(wk, WSCALE, e4)
    bqc = np.ascontiguousarray(bq.astype(np.float32).reshape(DIM, 1)) * WSCALE
    bkc = np.ascontiguousarray(bk.astype(np.float32).reshape(DIM, 1)) * WSCALE

    fdt = bf if mode == "bf16" else e4
    in_maps = []
    for b in range(B):
        m = {"wq": wqp, "wk": wkp, "bq": bqc, "bk": bkc}
        for nmk, f in (("f0", feat0[b]), ("f2", feat2[b])):
            fh = _pad_feat(f, fdt)
            m[nmk + "h"] = fh
            if mode == "fp8x2":
                res = (f.astype(np.float32)
                       - fh.astype(np.float32).reshape(DIM, FR, FC)[:, 1:97,
                                                                    1:97])
                m[nmk + "l"] = _pad_feat(res * LSCALE, e4)
        in_maps.append(m)

    trace = bool(int(os.environ.get("BASSFLOW_TRACE", "0")))
    res = bass_utils.run_bass_kernel_spmd(nc, in_maps, core_ids=list(range(B)),
                                          trace=trace)
    if trace:
        print(f"HW exec time: {res.exec_time_ns} ns "
              f"(mean {res.mean_exec_time_ns})")
        if res.instructions_and_trace:
            print("trace path:", res.instructions_and_trace[1])
    raw = np.stack([res.results[b]["corr"] for b in range(B)])
    raw = raw.astype(np.float32) * (1.0 / (WSCALE * WSCALE))
    # raw: [B, g=d*2+rxi, part, wx*24+wy*2+eo, 32]
    # E matmul: parts 0:64 = A=(ry0) win(wy,wx) k 32:64;
    #           parts 64:128 = B=(ry4) win(wy,wx) k 0:32
    # O matmul: parts 0:64 = B win(wy-1,wx) k 32:64;
    #           parts 64:128 = A win(wy,wx) k 0:32
    raw = raw.reshape(B, 4, 2, P * P, S2, S1, 2, 32)
    corr = np.empty((B, NV, NW, P * P, P * P), np.float32)

    def wqk(x):
        """[B, 64q, wx, wy, 32] -> [B, wy, wx, q, 32]."""
        return x.transpose(0, 3, 2, 1, 4)

    for d in range(2):
        for rxi in range(2):
            g = d * 2 + rxi
            E = raw[:, g, :, :, :, :, 0]        # [B, half, 64q, wx, wy, 32]
            O = raw[:, g, :, :, :, :, 1]
            vdA = rxi * 2 + d
            vdB = (2 + rxi) * 2 + d
            corr[:, vdA, :, :, 32:64] = wqk(E[:, 0]).reshape(B, NW, 64, 32)
            corr[:, vdA, :, :, 0:32] = wqk(O[:, 1]).reshape(B, NW, 64, 32)
            corr[:, vdB, :, :, 0:32] = wqk(E[:, 1]).reshape(B, NW, 64, 32)
            corr[:, vdB, :, :, 32:64] = np.roll(
                wqk(O[:, 0]), -1, axis=1).reshape(B, NW, 64, 32)
    return corr


# --------------------------------------------------------------------------
# Host tail: bias/mask + softmax flow pipeline + splice + bilinear upsample
# (numpy port of the reference; ~1% of total FLOPs)
# --------------------------------------------------------------------------

def _bias_index():
    coords = np.stack(np.meshgrid(np.arange(P), np.arange(P),
                                  indexing='ij')).reshape(2, -1)
    rel = (coords[:, :, None] - coords[:, None, :]).transpose(1, 2, 0).copy()
    rel[..., 0] += P - 1
    rel[..., 1] += P - 1
    rel[..., 0] *= 2 * P - 1
    return rel.sum(-1).reshape(-1)


def _pos():
    r = np.arange(P, dtype=np.float32)
    yy, xx = np.meshgrid(r, r, indexing='ij')
    return np.stack([xx, yy])[None].reshape(1, 2, P * P)


def _make_mask(Hp, Wp, sh, sw):
    m = np.zeros((Hp, Wp))
    hs = ((slice(0, -sh * 2), slice(-sh * 2, -sh), slice(-sh, None))
          if sh else (slice(None),))
    ws = ((slice(0, -sw * 2), slice(-sw * 2, -sw), slice(-sw, None))
          if sw else (slice(None),))
    cnt = 0
    for a in hs:
        for b in ws:
            m[a, b] = cnt
            cnt += 1
    win = m.reshape(Hp // P, P, Wp // P, P).transpose(0, 2, 1, 3).reshape(-1, P * P)
    d = win[:, None, :] - win[:, :, None]
    return np.where(d != 0, -10000.0, 0.0).astype(np.float32)


def _softmax(x, axis):
    m = np.max(x, axis=axis, keepdims=True)
    e = np.exp(x - m)
    return e / np.sum(e, axis=axis, keepdims=True)


_MID_IDX = None


def _mid_gather():
    """c[b, (j,k), (h2,w2)] = corr[b, (j+3-h2, k+3-w2), (h2,w2)] (0 if invalid)."""
    global _MID_IDX
    if _MID_IDX is None:
        j, k, h2, w2 = np.meshgrid(np.arange(9), np.arange(9), np.arange(P),
                                   np.arange(P), indexing='ij')
        qy = j + 3 - h2
        qx = k + 3 - w2
        valid = (qy >= 0) & (qy < P) & (qx >= 0) & (qx < P)
        qidx = np.clip(qy, 0, P - 1) * P + np.clip(qx, 0, P - 1)
        kidx = h2 * P + w2
        _MID_IDX = (qidx.reshape(81, 64), kidx.reshape(81, 64),
                    valid.reshape(81, 64))
    return _MID_IDX


def _flow_mid(corr, pos):
    bw = corr.shape[0]
    qidx, kidx, valid = _mid_gather()
    c = corr[:, qidx, kidx] * valid[None]          # (bw, 81, 64)
    n = P + 1
    r = np.arange(0.0, P - 0.5, 0.5)
    yy, xx = np.meshgrid(r, r, indexing='ij')
    CH = P // 2 - 1
    base = np.stack([xx, yy])[None][:, :, CH:2 * P - 1 - CH, CH:2 * P - 1 - CH]
    base = base.reshape(1, 2, n * n).astype(np.float32)
    flow = pos[:, :, None, :] - base[:, :, :, None]          # (1,2,81,64)
    smax = _softmax(c, axis=2)
    fl = np.einsum('bmk,cmk->bcm', smax, flow[0]).reshape(bw, 2, n, n)
    cr = np.sum(c * smax, axis=2).reshape(bw, 1, n, n)
    corr4 = np.concatenate([cr[:, :, :-1, :-1], cr[:, :, :-1, 1:],
                            cr[:, :, 1:, :-1], cr[:, :, 1:, 1:]], axis=1)
    flow4 = np.concatenate([fl[:, :, :-1, :-1], fl[:, :, :-1, 1:],
                            fl[:, :, 1:, :-1], fl[:, :, 1:, 1:]], axis=1)
    corr4 = corr4.transpose(0, 2, 3, 1).reshape(bw, P * P, 4)
    flow4 = flow4.reshape(bw, 4, 2, P, P).transpose(0, 2, 3, 4, 1)
    flow4 = flow4.reshape(bw, 2, P * P, 4) * 2
    smax2 = _softmax(corr4, axis=2)
    out = np.sum(flow4 * smax2[:, None], axis=3)
    return out.reshape(bw, 2, P, P).astype(np.float32)


def _flow_bsd(corr, pos):
    cut = P // 4
    bw = corr.shape[0]
    c = corr.reshape(bw, P, P, P * P)[:, cut:P - cut, cut:P - cut, :]
    L = (P - 2 * cut) ** 2
    c = c.reshape(bw, L, P * P)
    base = _pos().reshape(1, 2, P, P)[:, :, cut:P - cut, cut:P - cut]
    base = base.reshape(1, 2, L)
    flow = pos[:, :, None, :] - base[:, :, :, None]
    smax = _softmax(c, axis=2)
    out = np.einsum('blk,clk->bcl', smax, flow[0])
    return out.reshape(bw, 2, P - 2 * cut, P - 2 * cut).astype(np.float32)


def _splice(f00, f01, f10, f11, factor, Ho, Wo):
    f = np.concatenate([np.concatenate([f00, f01], axis=3),
                        np.concatenate([f10, f11], axis=3)], axis=2)
    bs, kk, hh, ww = f.shape
    b = bs // (S1 * S2)
    f = f.reshape(b, S1, S2, kk, hh, ww).transpose(0, 3, 1, 4, 2, 5)
    f = f.reshape(b, kk, S1 * hh, S2 * ww)
    sft = (P // 4) * factor
    f = np.roll(f, (sft, sft), axis=(2, 3))
    return f[:, :, :Ho * factor, :Wo * factor]


def _resize_mat(in_size, out_size):
    scale = out_size / in_size
    sample = (np.arange(out_size) + 0.5) / scale - 0.5
    x = np.abs(sample[None, :] - np.arange(in_size)[:, None])
    w = np.maximum(0.0, 1.0 - x)
    tot = w.sum(0, keepdims=True)
    return (w / np.where(tot == 0, 1.0, tot)).astype(np.float32)


def _up(x, f):
    b, c, h, w = x.shape
    My = _resize_mat(h, h * f)
    Mx = _resize_mat(w, w * f)
    y = np.einsum('bchw,hH->bcHw', x, My)
    y = np.einsum('bcHw,wW->bcHW', y, Mx)
    return (y * f).astype(np.float32)


def _host_flow(corr_raw, bias_table):
    """corr_raw: (B, NV, NW, 64, 64) raw q.k^T dot products."""
    bias = bias_table.astype(np.float32)[_bias_index()].reshape(
        P * P, P * P, 1).transpose(2, 0, 1)          # (1,64,64)
    pos = _pos()
    masks = {}
    for v, (sh, sw) in enumerate(((0, 0), (0, 4), (4, 0), (4, 4))):
        masks[v] = _make_mask(H, W, sh, sw) if (sh or sw) else None

    f1 = {}
    f0 = {}
    for v in range(4):
        for d in range(2):
            c = corr_raw[:, v * 2 + d].reshape(B * NW, 64, 64) * SCALE + bias
            if masks[v] is not None:
                c = (c.reshape(B, NW, 64, 64) + masks[v][None]).reshape(
                    B * NW, 64, 64)
            f1[(v, d)] = _flow_mid(c, pos)
            f0[(v, d)] = _flow_bsd(c, pos)

    # direction 0: (q0,k2) -> flow12 (mid), flow02 (bsd)
    # direction 1: (q2,k0) -> flow10 (mid), flow20 (bsd)
    flow12 = _splice(f1[(0, 0)], f1[(1, 0)], f1[(2, 0)], f1[(3, 0)], 2, H, W)
    flow02 = _splice(f0[(0, 0)], f0[(1, 0)], f0[(2, 0)], f0[(3, 0)], 1, H, W)
    flow10 = _splice(f1[(0, 1)], f1[(1, 1)], f1[(2, 1)], f1[(3, 1)], 2, H, W)
    flow20 = _splice(f0[(0, 1)], f0[(1, 1)], f0[(2, 1)], f0[(3, 1)], 1, H, W)
    fh, ff = UP // 2, UP
    return (_up(flow10, fh), _up(flow12, fh), _up(flow02, ff), _up(flow20, ff))


def kernel(feat0, feat2, wq, bq, wk, bk, bias_table):
    corr_raw = _run_device(np.asarray(feat0), np.asarray(feat2),
                           np.asarray(wq), np.asarray(bq),
                           np.asarray(wk), np.asarray(bk))
    return _host_flow(corr_raw, np.asarray(bias_table))



# revision 37
# speedup vs baseline: 1.2773x; 1.2773x over previous
"""Trainium2 Bass kernel for nn_BasicFlow (sparse window attention flow).

Sharding: pure data-parallel over batch B=8 -> one image pair per NeuronCore.

Device (per core), v2 restructure:
  - PE warm-up accumulation group during the input-DMA wait (p-state ramp).
  - 4x conv3x3 (128->128ch, 96x96) as fp8e4m3 DoubleRow matmuls over
    host-pre-padded flat feature maps. Feature repr per image:
      x2 : hi/lo split (fh = fp8(f), fl = fp8(32*(f-fh))), 9 DR matmuls per
           4-row strip (slots = one tap's 32W/W pair) -> features exact.
      1s : single fp8 set, 5 DR matmuls per strip (slots = two taps' 32W).
    MODE "x2" uses x2 for both images; "x21s" uses 1s for feat0 (error
    measured 1.43e-2/1.71e-2 on the cpu/axon RNG realizations, gate 2e-2).
  - q convs drain into half-block window layouts qhb[c, phase, wx, hb, 32]
    (hb h = img rows 4h-4..4h, 26 entries incl. two wrap dups; phase = col
    shift 0/4 written directly from psum, no separate shift copies).
  - corr: per (dir, phase, r, wx) ONE matmul: stationary = 96 contiguous q
    (12-row span x 8 cols = exactly the rows that need k row-group r),
    moving = k[4 rows x 8 cols] fp16 -> out psum [96, 32]. Each k pixel is
    streamed once per (dir, phase); the y-axis window overlap is deduped in
    the output (25% fewer bytes than the A|B pair scheme).
  - corr groups are interleaved INTO the k-conv strip loops (corr row-group
    r emitted after strip r+1), so corr PE/drains/DMA hide inside the convs.
  - fp16 (not bf16) for q/k/corr storage: same cost, 8x finer mantissa.
  - psum drains spread greedily across Act/DVE/Pool by modeled cost.
Host: window reassembly + softmax/flow/splice/bilinear tail (numpy).
"""

import os

# recover wedged NeuronCores at NRT init (observed transient
# NRT_EXEC_UNIT_UNRECOVERABLE; reset-on-load clears it)
os.environ.setdefault("NEURON_RT_RESET_CORES", "1")

import numpy as np
import ml_dtypes

import concourse.bass as bass
import concourse.bacc as bacc
import concourse.tile as tile
import concourse.mybir as mybir
from concourse import bass_utils
from concourse.bass import AP

F32 = mybir.dt.float32
FP16 = mybir.dt.float16
FP8 = mybir.dt.float8e4

B = 8
DIM = 128
H = W = 96
P = 8
UP = 4
SCALE = DIM ** -0.5
S1 = S2 = H // P          # 12 windows per axis
NW = S1 * S2              # 144 windows
NV = 8                    # 4 shift variants x 2 directions

# padded feature map: row 0 + 96 image rows + row 97 border + 3 zero tail
# rows (DoubleRow zero-slot reads); cols: 1 border + 96 + 1 border
FR = 101                  # fpad rows
FC = 98                   # fpad cols
NRT = H // 4              # 24 strips of 4 output rows
NHB = 26                  # q half-blocks: 24 + 2 wrap dups
NR = 24                   # corr k row-groups

WSCALE = 32.0             # weight pre-scale; corr comes back x WSCALE^2

# conv taps as 1-set DoubleRow pairs: (slot0 tap, slot1 tap, ifmap delta)
# tap (dy, dx) reads flat offset dy*FC + dx relative to the strip base.
# pair 4 slot1 is a zero-weight dummy (reads in-bounds zero-padded rows).
_PAIRS = [((0, 0), (0, 1)), ((0, 2), (1, 0)), ((1, 1), (1, 2)),
          ((2, 0), (2, 1)), ((2, 2), None)]
_PAIR_OFF = [dy * FC + dx for ((dy, dx), _) in _PAIRS]
_PAIR_D = []
for (t0, t1) in _PAIRS:
    if t1 is None:
        _PAIR_D.append(FC)    # dummy slot: any in-bounds stride
    else:
        _PAIR_D.append((t1[0] * FC + t1[1]) - (t0[0] * FC + t0[1]))

# MODE: "x2" = both feature maps exact (hi/lo);  "x21s" = feat0 single-set
MODE = os.environ.get("BASSFLOW_MODE", "x21s")

_COMPILED = {}


# --------------------------------------------------------------------------
# Device kernel
# --------------------------------------------------------------------------

class DrainBalancer:
    """Greedy psum-drain scheduler across Act/DVE/Pool by modeled ns cost."""

    RATE = {"act": 0.833, "dve": 1.04, "pool": 1.39}
    OVH = {"act": 190.0, "dve": 130.0, "pool": 100.0}

    def __init__(self, nc):
        self.nc = nc
        self.load = {"act": 0.0, "dve": 0.0, "pool": 0.0}

    def _pick(self, elems, allowed):
        best, bcost = None, None
        for e in allowed:
            c = self.load[e] + elems * self.RATE[e] + self.OVH[e]
            if bcost is None or c < bcost:
                best, bcost = e, c
        self.load[best] = bcost
        return best

    def drain(self, dst, src, bias=None, engines=None, force=None):
        # GPSIMD (pool) cannot access PSUM: psum-sourced ops go Act/DVE only
        if engines is None:
            engines = (("act", "dve") if src.space == bass.MemorySpace.PSUM
                       else ("act", "dve", "pool"))
        elems = 1
        for _, n in dst.ap[1:]:
            elems *= n
        if force is not None:
            eng = force
            self.load[eng] += elems * self.RATE[eng] + self.OVH[eng]
        else:
            eng = self._pick(elems, engines)
        nc = self.nc
        if bias is None:
            if eng == "act":
                nc.scalar.copy(dst, src)
            elif eng == "dve":
                nc.vector.tensor_copy(dst, src)
            else:
                nc.gpsimd.tensor_copy(dst, src)
        else:
            if eng == "act":
                nc.scalar.activation(
                    dst, src, mybir.ActivationFunctionType.Identity, bias=bias)
            elif eng == "dve":
                nc.vector.tensor_scalar_add(dst, src, bias)
            else:
                nc.gpsimd.tensor_scalar_add(dst, src, bias)


def _build_device(mode):
    assert mode in ("x2", "x21s")
    f0mode = "1s" if mode == "x21s" else "x2"
    f2mode = "x2"

    nc = bacc.Bacc("TRN2", target_bir_lowering=False, debug=False,
                   num_devices=8)

    nsl = {"f0": 2 if f0mode == "x2" else 1, "f2": 2 if f2mode == "x2" else 1}
    f_d = {}
    f_d["f0h"] = nc.dram_tensor("f0h", [DIM, FR * FC], FP8,
                                kind="ExternalInput")
    if f0mode == "x2":
        f_d["f0l"] = nc.dram_tensor("f0l", [DIM, FR * FC], FP8,
                                    kind="ExternalInput")
    f_d["f2h"] = nc.dram_tensor("f2h", [DIM, FR * FC], FP8,
                                kind="ExternalInput")
    if f2mode == "x2":
        f_d["f2l"] = nc.dram_tensor("f2l", [DIM, FR * FC], FP8,
                                    kind="ExternalInput")

    # weight forms: hl = per-tap (32W, W) hi/lo slots; pr = tap pairs (32W,32W)
    w_d = {}
    need_hl_q = f2mode == "x2"          # q2 conv runs on f2
    need_pr_q = f0mode == "1s"          # q0 conv runs on f0
    if need_hl_q or f0mode == "x2":
        w_d["wq_hl"] = nc.dram_tensor("wq_hl", [DIM, 9, 2, DIM], FP8,
                                      kind="ExternalInput")
        w_d["wk_hl"] = nc.dram_tensor("wk_hl", [DIM, 9, 2, DIM], FP8,
                                      kind="ExternalInput")
    if need_pr_q:
        w_d["wq_pr"] = nc.dram_tensor("wq_pr", [DIM, 5, 2, DIM], FP8,
                                      kind="ExternalInput")
        w_d["wk_pr"] = nc.dram_tensor("wk_pr", [DIM, 5, 2, DIM], FP8,
                                      kind="ExternalInput")
    bq_d = nc.dram_tensor("bq", [DIM, 1], F32, kind="ExternalInput")
    bk_d = nc.dram_tensor("bk", [DIM, 1], F32, kind="ExternalInput")
    dbg = bool(int(os.environ.get("BASSFLOW_DEBUG", "0")))
    dbg_d = {}
    if dbg:
        for nm2 in ("qhb0", "qhb2"):
            dbg_d[nm2] = nc.dram_tensor("dbg_" + nm2, [DIM, 2 * S2 * NHB * 32],
                                        FP16, kind="ExternalOutput")
        for nm2 in ("k2e", "k0e"):
            dbg_d[nm2] = nc.dram_tensor("dbg_" + nm2, [DIM, H * W],
                                        FP16, kind="ExternalOutput")
    # corr out: [g = dir*2 + phase, 96 q-parts, r*12*32 + wx*32 + k]
    corr_d = nc.dram_tensor("corr", [4, 96, NR * S2 * 32], FP16,
                            kind="ExternalOutput")

    with tile.TileContext(nc) as tc:
        with (
            tc.tile_pool(name="const", bufs=1) as constp,
            tc.tile_pool(name="big", bufs=1) as bigp,
            tc.tile_pool(name="qk", bufs=1) as qkp,
            tc.tile_pool(name="st", bufs=4) as stp,
            tc.tile_pool(name="psum", bufs=1, space="PSUM") as psump,
        ):
            bal = DrainBalancer(nc)

            w_sb = {}
            for nm, d in w_d.items():
                w_sb[nm] = constp.tile(list(d.shape), FP8, tag=nm, name=nm)
            bq_sb = constp.tile([DIM, 1], F32, tag="bq")
            bk_sb = constp.tile([DIM, 1], F32, tag="bk")

            fpads = {}
            for nmk in ("f0", "f2"):
                fpads[nmk] = bigp.tile([DIM, nsl[nmk], FR * FC], FP8,
                                       tag="fp" + nmk, name="fp" + nmk)

            # q-side conv weights + bias on the Act HWDGE queue; ALL feature
            # chunks + k weights on the SP HWDGE queue in consumption order
            # (one ordered queue: f2 cannot get ahead of the f0 chunks the
            # first conv is consuming). corr output reuses the SP queue too.
            q0w = "wq_pr" if f0mode == "1s" else "wq_hl"
            nc.scalar.dma_start(w_sb[q0w][:], w_d[q0w][:])
            nc.scalar.dma_start(bq_sb[:], bq_d[:])
            for nm in w_sb:
                if nm in (q0w, "wk_hl", "wk_pr"):
                    continue
                nc.scalar.dma_start(w_sb[nm][:], w_d[nm][:])

            CHUNKS = [(0, 8), (8, 20), (20, 33), (33, 46), (46, 60),
                      (60, 74), (74, 88), (88, FR)]
            f0srcs = [("f0", 0, f_d["f0h"])]
            if f0mode == "x2":
                f0srcs.append(("f0", 1, f_d["f0l"]))
            f2srcs = [("f2", 0, f_d["f2h"])]
            if f2mode == "x2":
                f2srcs.append(("f2", 1, f_d["f2l"]))
            for (r0, r1) in CHUNKS:
                for nmk, sl, src_d in f0srcs:
                    nc.sync.dma_start(fpads[nmk][:, sl, r0 * FC:r1 * FC],
                                      src_d[:, r0 * FC:r1 * FC])
            # phase order is q0, k0, q2+d1, k2+d0: k0's weights right
            # after f0, then f2, then k2's weights
            if "wk_pr" in w_sb:
                nc.sync.dma_start(w_sb["wk_pr"][:], w_d["wk_pr"][:])
            else:
                nc.sync.dma_start(w_sb["wk_hl"][:], w_d["wk_hl"][:])
            nc.sync.dma_start(bk_sb[:], bk_d[:])
            F2CHUNKS = [(0, 20), (20, 48), (48, 76), (76, FR)]
            for ci, (r0, r1) in enumerate(F2CHUNKS):
                for nmk, sl, src_d in f2srcs:
                    nc.sync.dma_start(fpads[nmk][:, sl, r0 * FC:r1 * FC],
                                      src_d[:, r0 * FC:r1 * FC])
            if "wk_hl" in w_sb and "wk_pr" in w_sb:
                nc.sync.dma_start(w_sb["wk_hl"][:], w_d["wk_hl"][:])

            # k extended fp16 maps (wrap cols) and q half-block layouts
            kexts = {1: qkp.tile([DIM, H, W], FP16, tag="k2e", name="k2e"),
                     3: qkp.tile([DIM, H, W], FP16, tag="k0e", name="k0e")}
            qhbs = {0: qkp.tile([DIM, 2, S2, NHB, 32], FP16, tag="qhb0",
                                name="qhb0"),
                    2: qkp.tile([DIM, 2, S2, NHB, 32], FP16, tag="qhb2",
                                name="qhb2")}

            # PE p-state warm-up during the input-DMA wait (shares the conv
            # psum tag so the conv strips get a 3-deep rotation)
            PSW = 4 * FC              # conv psum strip width
            wps = psump.tile([DIM, 512], F32, tag="ps", bufs=4, name="wps")
            cone = nc.const_aps.scalar_like(1.0, wps[:, 0:1], mybir.dt.bfloat16)
            warm_l = cone.broadcast_to([DIM, DIM])
            warm_r = cone.broadcast_to([DIM, 512])
            # preload the Act function table (1.3us) during the DMA wait so
            # the first conv drains are not blocked behind it
            nc.vector.memset(kexts[1][:, 0, 0:1], 0.0)
            nc.scalar.copy(kexts[1][:, 0, 1:2], kexts[1][:, 0, 0:1])
            NWARM = 6
            for i in range(NWARM):
                nc.tensor.matmul(wps[:], warm_l, warm_r,
                                 start=(i == 0), stop=(i == NWARM - 1))

            def hilo_ap(fp_sb, rt, t):
                """x2: DoubleRow slots = same tap's hi and lo terms."""
                base = fp_sb[:]
                pitch = base.ap[0][0]
                dy, dx = divmod(t, 3)
                off = rt * 4 * FC + dy * FC + dx
                return AP(tensor=base.tensor, offset=off,
                          ap=[[pitch, DIM], [FR * FC, 2], [1, PSW]])

            def pair_ap(fp_sb, rt, pr):
                """1s: DoubleRow slots = two taps of the single fp8 set."""
                base = fp_sb[:]
                pitch = base.ap[0][0]
                off = rt * 4 * FC + _PAIR_OFF[pr]
                return AP(tensor=base.tensor, offset=off,
                          ap=[[pitch, DIM], [_PAIR_D[pr], 2], [1, PSW]])

            def conv_mms(ps, fkey, fmode, wname, rt):
                fp_sb = fpads[fkey]
                if fmode == "x2":
                    for t in range(9):
                        nc.tensor.matmul(
                            ps[:], w_sb[wname][:, t], hilo_ap(fp_sb, rt, t),
                            start=(t == 0), stop=(t == 8),
                            perf_mode=mybir.MatmulPerfMode.DoubleRow)
                else:
                    for i in range(5):
                        nc.tensor.matmul(
                            ps[:], w_sb[wname][:, i], pair_ap(fp_sb, rt, i),
                            start=(i == 0), stop=(i == 4),
                            perf_mode=mybir.MatmulPerfMode.DoubleRow)

            def psrc(ps, dims, off):
                """Strided view into the [128, 4*FC] psum strip."""
                base = ps[:]
                pitch = base.ap[0][0]
                return AP(tensor=base.tensor, offset=base.offset + off,
                          ap=[[pitch, DIM]] + dims)

            def conv_q(dirn, fkey, fmode, wname, first23=False, corr=None):
                qhb = qhbs[dirn]
                qv = qhb.rearrange("p f w h (r l) -> p f w h r l", l=P)
                # first23: run strip 23 first (its features must already be
                # resident) so the wrap dups complete long before the corr
                # phase that follows needs them
                rts = ([NRT - 1] + list(range(NRT - 1))) if first23 \
                    else list(range(NRT))
                for i, rt in enumerate(rts):
                    ps = psump.tile([DIM, PSW], F32, tag="ps", bufs=4,
                                    name="ps")
                    conv_mms(ps, fkey, fmode, wname, rt)
                    hb = rt + 1
                    # phase 0 drained from psum (Act/DVE); phase 4 is built
                    # from phase 0 in SBUF (Pool-eligible copies)
                    bal.drain(qv[:, 0, :, hb],
                              psrc(ps, [[P, S2], [FC, 4], [1, P]], 0),
                              bq_sb[:])
                    bal.drain(qv[:, 1, :, hb, :, 0:4], qv[:, 0, :, hb, :, 4:8])
                    bal.drain(qv[:, 1, 0:S2 - 1, hb, :, 4:8],
                              qv[:, 0, 1:S2, hb, :, 0:4])
                    bal.drain(qv[:, 1, S2 - 1, hb, :, 4:8],
                              qv[:, 0, 0, hb, :, 0:4])
                    # wrap dups: hb 0 = hb 24 (rows 92..96), hb 25 = hb 1
                    if rt == NRT - 1:
                        bal.drain(qhb[:, :, :, 0, :], qhb[:, :, :, 24, :],
                                  force="act")
                    if rt == 0:
                        bal.drain(qhb[:, :, :, 25, :], qhb[:, :, :, 1, :],
                                  force="dve")
                    if corr is not None and i >= 6:
                        d, kext = corr
                        corr_r(d, i - 6, qhb, kext,
                               engs=(("dve", "act")[i % 2],
                                     ("act", "dve")[i % 2]))
                if corr is not None:
                    d, kext = corr
                    for r in range(NRT - 6, NRT):
                        corr_r(d, r, qhb, kext, engs=("act", "dve"))

            # ---- corr machinery -------------------------------------------
            st_state = {}

            # st DMA chunks: 4 row-groups each; the final TWO row-groups
            # (22, 23) are handled separately with one merged two-phase DMA
            NTAIL = 2
            ST_STARTS = [0, 4, 8, 12, 16, 20]
            ST_NEXT = {s: (ST_STARTS + [NR - NTAIL])[i + 1]
                       for i, s in enumerate(ST_STARTS)}

            def corr_r(d, r, qhb, kext, engs=(None, None)):
                """One k row-group r of both phases of direction d."""
                tail = r >= NR - NTAIL
                if r == NR - NTAIL:
                    # final row-groups: both phases in one tile, ONE merged
                    # DMA (shortest possible end-of-program chain)
                    st_state["tail"] = stp.tile(
                        [96, 2, NTAIL, S2, 32], FP16, tag="sttail",
                        bufs=2, name=f"sttail{d}")
                for pi in range(2):
                    g = d * 2 + pi
                    cps = psump.tile([96, S2, 32], F32, tag="cps", bufs=4,
                                     name="cps")
                    for wx in range(S2):
                        stat = qhb[:, pi, wx, r:r + 3, :]
                        c0x = 8 * wx + 4 * pi
                        if c0x + 8 <= W:
                            mov = kext[:, 4 * r:4 * r + 4, c0x:c0x + 8]
                            nc.tensor.matmul(cps[:, wx, :], stat, mov,
                                             start=True, stop=True)
                        else:
                            # wrapping window (phase 4, wx 11): two matmuls
                            # fill the two col-halves of the psum slice
                            cv = cps[:, wx, :].rearrange(
                                "p (r l) -> p r l", l=P)
                            nc.tensor.matmul(
                                cv[:, :, 0:4], stat,
                                kext[:, 4 * r:4 * r + 4, c0x:W],
                                start=True, stop=True)
                            nc.tensor.matmul(
                                cv[:, :, 4:8], stat,
                                kext[:, 4 * r:4 * r + 4, 0:4],
                                start=True, stop=True)
                    if tail:
                        bal.drain(st_state["tail"][:, pi, r - (NR - NTAIL)],
                                  cps[:], force=("act", "dve")[pi])
                        continue
                    if r in ST_NEXT:
                        st_state[g] = (r, stp.tile(
                            [96, ST_NEXT[r] - r, S2, 32], FP16,
                            tag="st", name=f"st{g}_{r}"))
                    c0, st = st_state[g]
                    bal.drain(st[:, r - c0], cps[:], force=engs[pi])
                    if r + 1 == ST_NEXT[c0]:
                        nc.sync.dma_start(
                            corr_d[g, :, c0 * S2 * 32:ST_NEXT[c0] * S2 * 32],
                            st[:].rearrange("p a b c -> p (a b c)"))
                if r == NR - 1:
                    base = corr_d[:]
                    dst = AP(tensor=base.tensor,
                             offset=(2 * d) * 96 * NR * S2 * 32
                             + (NR - NTAIL) * S2 * 32,
                             ap=[[NR * S2 * 32, 96], [96 * NR * S2 * 32, 2],
                                 [1, NTAIL * S2 * 32]])
                    nc.sync.dma_start(dst, st_state["tail"][:])

            def conv_k_corr(dirn, fkey, fmode, wname, d=None, qdirn=None):
                kext = kexts[dirn]
                ROT = ("act", "dve", "pool")
                for rt in range(NRT):
                    ps = psump.tile([DIM, PSW], F32, tag="ps", bufs=4,
                                    name="ps")
                    conv_mms(ps, fkey, fmode, wname, rt)
                    # one drain; cols 96:100 get junk, fixed by the small
                    # per-strip wrap drain (corr phase-4 wx 11 reads them)
                    kd = ("act", "dve")[rt % 2]
                    bal.drain(kext[:, 4 * rt:4 * rt + 4, :],
                              psrc(ps, [[FC, 4], [1, 96]], 0), bk_sb[:],
                              force=kd)
                    # corr lags its k strip by 3 so the strip's drain chain
                    # (sem + engine queue + exec) never stalls the PE
                    if d is not None and rt >= 3:
                        corr_r(d, rt - 3, qhbs[qdirn], kext,
                               engs=(("dve", "act")[rt % 2],
                                     ("act", "dve")[rt % 2]))
                if d is not None:
                    for r in (NRT - 3, NRT - 2, NRT - 1):
                        corr_r(d, r, qhbs[qdirn], kext, engs=("act", "dve"))

            # ---- schedule: q0, k0 (conv only), q2+d1, k2+d0 ----------------
            conv_q(0, "f0", f0mode, q0w)
            conv_k_corr(3, "f0", f0mode,
                        "wk_pr" if f0mode == "1s" else "wk_hl")
            conv_q(2, "f2", f2mode, "wq_hl" if f2mode == "x2" else "wq_pr",
                   first23=True, corr=(1, kexts[3]))
            conv_k_corr(1, "f2", f2mode,
                        "wk_hl" if f2mode == "x2" else "wk_pr", 0, 0)
            if dbg:
                for nm2, t in (("qhb0", qhbs[0]), ("qhb2", qhbs[2]),
                               ("k2e", kexts[1]), ("k0e", kexts[3])):
                    nc.sync.dma_start(
                        dbg_d[nm2][:],
                        t[:].rearrange("p ... -> p (...)")
                        if False else
                        AP(tensor=t[:].tensor, offset=0,
                           ap=[[t[:].ap[0][0], DIM],
                               [1, dbg_d[nm2].shape[1]]]))

    nc.compile()
    return nc


# --------------------------------------------------------------------------
# Host-side prep + run
# --------------------------------------------------------------------------

def _pack_weights_hilo(w, dt):
    """(out, in, 3, 3) -> (in, 9, 2, out); slot0 = 32W (hi), slot1 = W (lo,
    features pre-scaled x32)."""
    w = np.asarray(w, np.float32)
    pk = np.zeros((DIM, 9, 2, DIM), np.float32)
    for t in range(9):
        dy, dx = divmod(t, 3)
        pk[:, t, 0, :] = (WSCALE * w)[:, :, dy, dx].T
        pk[:, t, 1, :] = w[:, :, dy, dx].T
    return np.ascontiguousarray(pk).astype(dt)


def _pack_weights_pairs(w, dt):
    """(out, in, 3, 3) -> (in, 5, 2, out) DoubleRow tap pairs, x WSCALE."""
    w = np.asarray(w, np.float32) * WSCALE
    pk = np.zeros((DIM, 5, 2, DIM), np.float32)
    for p, (t0, t1) in enumerate(_PAIRS):
        pk[:, p, 0, :] = w[:, :, t0[0], t0[1]].T
        if t1 is not None:
            pk[:, p, 1, :] = w[:, :, t1[0], t1[1]].T
    return np.ascontiguousarray(pk).astype(dt)


def _pad_feat(f, dt):
    """f: (128, 96, 96) f32 -> flat (128, FR*FC) padded, quantized to dt."""
    fp = np.zeros((DIM, FR, FC), np.float32)
    fp[:, 1:97, 1:97] = f
    return np.ascontiguousarray(fp.reshape(DIM, FR * FC)).astype(dt)


def _run_device(feat0, feat2, wq, bq, wk, bk):
    mode = MODE
    if mode not in _COMPILED:
        _COMPILED[mode] = _build_device(mode)
    nc = _COMPILED[mode]
    f0mode = "1s" if mode == "x21s" else "x2"

    e4 = ml_dtypes.float8_e4m3
    base = {
        "bq": np.ascontiguousarray(bq.astype(np.float32).reshape(DIM, 1))
        * WSCALE,
        "bk": np.ascontiguousarray(bk.astype(np.float32).reshape(DIM, 1))
        * WSCALE,
    }
    base["wq_hl"] = _pack_weights_hilo(wq, e4)
    base["wk_hl"] = _pack_weights_hilo(wk, e4)
    if f0mode == "1s":
        base["wq_pr"] = _pack_weights_pairs(wq, e4)
        base["wk_pr"] = _pack_weights_pairs(wk, e4)

    in_maps = []
    for b in range(B):
        m = dict(base)
        for nmk, f, fm in (("f0", feat0[b], f0mode), ("f2", feat2[b], "x2")):
            f = f.astype(np.float32)
            fh = _pad_feat(f, e4)
            m[nmk + "h"] = fh
            if fm == "x2":
                res = (f - fh.astype(np.float32).reshape(DIM, FR, FC)
                       [:, 1:97, 1:97])
                m[nmk + "l"] = _pad_feat(res * WSCALE, e4)
        in_maps.append(m)

    trace = bool(int(os.environ.get("BASSFLOW_TRACE", "0")))
    res = bass_utils.run_bass_kernel_spmd(nc, in_maps, core_ids=list(range(B)),
                                          trace=trace)
    if trace:
        print(f"HW exec time: {res.exec_time_ns} ns "
              f"(mean {res.mean_exec_time_ns})")
        if res.instructions_and_trace:
            print("trace path:", res.instructions_and_trace[1])
    raw = np.stack([res.results[b]["corr"] for b in range(B)])
    raw = raw.astype(np.float32) * (1.0 / (WSCALE * WSCALE))
    # raw: [B, g = d*2 + pi, 96 q-parts, r, wx, 32 k]
    raw = raw.reshape(B, 4, 96, NR, S2, 32)
    corr = np.empty((B, NV, NW, P * P, P * P), np.float32)
    wy = np.arange(S1)
    for d in range(2):
        for pi in range(2):
            g = d * 2 + pi
            for ry4 in range(2):
                vd = (2 * ry4 + pi) * 2 + d
                rL = 2 * wy + ry4
                rU = (rL + 1) % NR
                lo = raw[:, g, 32:96][:, :, rL]    # [B, 64, wy, wx, 32]
                hi = raw[:, g, 0:64][:, :, rU]
                corr[:, vd, :, :, 0:32] = lo.transpose(0, 2, 3, 1, 4).reshape(
                    B, NW, 64, 32)
                corr[:, vd, :, :, 32:64] = hi.transpose(0, 2, 3, 1, 4).reshape(
                    B, NW, 64, 32)
    return corr


# --------------------------------------------------------------------------
# Host tail: bias/mask + softmax flow pipeline + splice + bilinear upsample
# (numpy port of the reference; ~1% of total FLOPs)
# --------------------------------------------------------------------------

def _bias_index():
    coords = np.stack(np.meshgrid(np.arange(P), np.arange(P),
                                  indexing='ij')).reshape(2, -1)
    rel = (coords[:, :, None] - coords[:, None, :]).transpose(1, 2, 0).copy()
    rel[..., 0] += P - 1
    rel[..., 1] += P - 1
    rel[..., 0] *= 2 * P - 1
    return rel.sum(-1).reshape(-1)


def _pos():
    r = np.arange(P, dtype=np.float32)
    yy, xx = np.meshgrid(r, r, indexing='ij')
    return np.stack([xx, yy])[None].reshape(1, 2, P * P)


def _make_mask(Hp, Wp, sh, sw):
    m = np.zeros((Hp, Wp))
    hs = ((slice(0, -sh * 2), slice(-sh * 2, -sh), slice(-sh, None))
          if sh else (slice(None),))
    ws = ((slice(0, -sw * 2), slice(-sw * 2, -sw), slice(-sw, None))
          if sw else (slice(None),))
    cnt = 0
    for a in hs:
        for b in ws:
            m[a, b] = cnt
            cnt += 1
    win = m.reshape(Hp // P, P, Wp // P, P).transpose(0, 2, 1, 3).reshape(-1, P * P)
    d = win[:, None, :] - win[:, :, None]
    return np.where(d != 0, -10000.0, 0.0).astype(np.float32)


def _softmax(x, axis):
    m = np.max(x, axis=axis, keepdims=True)
    e = np.exp(x - m)
    return e / np.sum(e, axis=axis, keepdims=True)


_MID_IDX = None


def _mid_gather():
    """c[b, (j,k), (h2,w2)] = corr[b, (j+3-h2, k+3-w2), (h2,w2)] (0 if invalid)."""
    global _MID_IDX
    if _MID_IDX is None:
        j, k, h2, w2 = np.meshgrid(np.arange(9), np.arange(9), np.arange(P),
                                   np.arange(P), indexing='ij')
        qy = j + 3 - h2
        qx = k + 3 - w2
        valid = (qy >= 0) & (qy < P) & (qx >= 0) & (qx < P)
        qidx = np.clip(qy, 0, P - 1) * P + np.clip(qx, 0, P - 1)
        kidx = h2 * P + w2
        _MID_IDX = (qidx.reshape(81, 64), kidx.reshape(81, 64),
                    valid.reshape(81, 64))
    return _MID_IDX


def _flow_mid(corr, pos):
    bw = corr.shape[0]
    qidx, kidx, valid = _mid_gather()
    c = corr[:, qidx, kidx] * valid[None]          # (bw, 81, 64)
    n = P + 1
    r = np.arange(0.0, P - 0.5, 0.5)
    yy, xx = np.meshgrid(r, r, indexing='ij')
    CH = P // 2 - 1
    base = np.stack([xx, yy])[None][:, :, CH:2 * P - 1 - CH, CH:2 * P - 1 - CH]
    base = base.reshape(1, 2, n * n).astype(np.float32)
    flow = pos[:, :, None, :] - base[:, :, :, None]          # (1,2,81,64)
    smax = _softmax(c, axis=2)
    fl = np.einsum('bmk,cmk->bcm', smax, flow[0]).reshape(bw, 2, n, n)
    cr = np.sum(c * smax, axis=2).reshape(bw, 1, n, n)
    corr4 = np.concatenate([cr[:, :, :-1, :-1], cr[:, :, :-1, 1:],
                            cr[:, :, 1:, :-1], cr[:, :, 1:, 1:]], axis=1)
    flow4 = np.concatenate([fl[:, :, :-1, :-1], fl[:, :, :-1, 1:],
                            fl[:, :, 1:, :-1], fl[:, :, 1:, 1:]], axis=1)
    corr4 = corr4.transpose(0, 2, 3, 1).reshape(bw, P * P, 4)
    flow4 = flow4.reshape(bw, 4, 2, P, P).transpose(0, 2, 3, 4, 1)
    flow4 = flow4.reshape(bw, 2, P * P, 4) * 2
    smax2 = _softmax(corr4, axis=2)
    out = np.sum(flow4 * smax2[:, None], axis=3)
    return out.reshape(bw, 2, P, P).astype(np.float32)


def _flow_bsd(corr, pos):
    cut = P // 4
    bw = corr.shape[0]
    c = corr.reshape(bw, P, P, P * P)[:, cut:P - cut, cut:P - cut, :]
    L = (P - 2 * cut) ** 2
    c = c.reshape(bw, L, P * P)
    base = _pos().reshape(1, 2, P, P)[:, :, cut:P - cut, cut:P - cut]
    base = base.reshape(1, 2, L)
    flow = pos[:, :, None, :] - base[:, :, :, None]
    smax = _softmax(c, axis=2)
    out = np.einsum('blk,clk->bcl', smax, flow[0])
    return out.reshape(bw, 2, P - 2 * cut, P - 2 * cut).astype(np.float32)


def _splice(f00, f01, f10, f11, factor, Ho, Wo):
    f = np.concatenate([np.concatenate([f00, f01], axis=3),
                        np.concatenate([f10, f11], axis=3)], axis=2)
    bs, kk, hh, ww = f.shape
    b = bs // (S1 * S2)
    f = f.reshape(b, S1, S2, kk, hh, ww).transpose(0, 3, 1, 4, 2, 5)
    f = f.reshape(b, kk, S1 * hh, S2 * ww)
    sft = (P // 4) * factor
    f = np.roll(f, (sft, sft), axis=(2, 3))
    return f[:, :, :Ho * factor, :Wo * factor]


def _resize_mat(in_size, out_size):
    scale = out_size / in_size
    sample = (np.arange(out_size) + 0.5) / scale - 0.5
    x = np.abs(sample[None, :] - np.arange(in_size)[:, None])
    w = np.maximum(0.0, 1.0 - x)
    tot = w.sum(0, keepdims=True)
    return (w / np.where(tot == 0, 1.0, tot)).astype(np.float32)


def _up(x, f):
    b, c, h, w = x.shape
    My = _resize_mat(h, h * f)
    Mx = _resize_mat(w, w * f)
    y = np.einsum('bchw,hH->bcHw', x, My)
    y = np.einsum('bcHw,wW->bcHW', y, Mx)
    return (y * f).astype(np.float32)


def _host_flow(corr_raw, bias_table):
    """corr_raw: (B, NV, NW, 64, 64) raw q.k^T dot products."""
    bias = bias_table.astype(np.float32)[_bias_index()].reshape(
        P * P, P * P, 1).transpose(2, 0, 1)          # (1,64,64)
    pos = _pos()
    masks = {}
    for v, (sh, sw) in enumerate(((0, 0), (0, 4), (4, 0), (4, 4))):
        masks[v] = _make_mask(H, W, sh, sw) if (sh or sw) else None

    f1 = {}
    f0 = {}
    for v in range(4):
        for d in range(2):
            c = corr_raw[:, v * 2 + d].reshape(B * NW, 64, 64) * SCALE + bias
            if masks[v] is not None:
                c = (c.reshape(B, NW, 64, 64) + masks[v][None]).reshape(
                    B * NW, 64, 64)
            f1[(v, d)] = _flow_mid(c, pos)
            f0[(v, d)] = _flow_bsd(c, pos)

    # direction 0: (q0,k2) -> flow12 (mid), flow02 (bsd)
    # direction 1: (q2,k0) -> flow10 (mid), flow20 (bsd)
    flow12 = _splice(f1[(0, 0)], f1[(1, 0)], f1[(2, 0)], f1[(3, 0)], 2, H, W)
    flow02 = _splice(f0[(0, 0)], f0[(1, 0)], f0[(2, 0)], f0[(3, 0)], 1, H, W)
    flow10 = _splice(f1[(0, 1)], f1[(1, 1)], f1[(2, 1)], f1[(3, 1)], 2, H, W)
    flow20 = _splice(f0[(0, 1)], f0[(1, 1)], f0[(2, 1)], f0[(3, 1)], 1, H, W)
    fh, ff = UP // 2, UP
    return (_up(flow10, fh), _up(flow12, fh), _up(flow02, ff), _up(flow20, ff))


def kernel(feat0, feat2, wq, bq, wk, bk, bias_table):
    corr_raw = _run_device(np.asarray(feat0), np.asarray(feat2),
                           np.asarray(wq), np.asarray(bq),
                           np.asarray(wk), np.asarray(bk))
    return _host_flow(corr_raw, np.asarray(bias_table))
